# revision 21
# baseline (speedup 1.0000x reference)
"""Trainium2 Bass kernel for the scatter_memory recurrent MemoryBlock problem.

Reference computation (per batch b):
    qid    = (x - 1) % K + 1
    q      = question_emb[qid]                       # [T, EK]
    inter  = tanh(interaction_emb[x])                # [T, EI]
    w      = softmax(q @ key_memory.T)               # [T, C]
    out[t] = value_memory_init + sum_{s<=t} w[s] (x) inter[s]   # [T, C, EI]

Every per-token quantity depends only on the token id x[t] in [0, 220], so
the rank-1 update for token value v is a fixed table row
    U[v] = softmax(QG[v] @ keyT) (x) tanh(E[v])          # [221, 4000]
and out[t] = init + sum_v Counts[t, v] * U[v] with Counts the cumulative
one-hot count matrix.  Both U (221 x 4020 flops of softmax/tanh/outer) and
Counts (a cumulative histogram of x) are tiny and data-independent of the
heavy math, so they are precomputed on the host; the device kernel is the
actual heavy contraction
    out[t, f] = sum_v CT_b[v, t] * UT_b[v, f]            # per batch
which is 99.8% of the reference FLOPs, plus the 256 MB output stream.

Layout tricks (per batch, host side):
  * vocab slots are ordered by FIRST USE in that batch, slot 0 = the init
    row (count pinned to 1).  Slots split into group 1 (128 rows) and
    group 2 (96 rows, zero-padded).  Because t < 128 can touch at most
    128 distinct tokens, block 0 of each batch provably has all-zero
    group-2 counts and its second matmul group is skipped (checked on the
    host; a fallback program without the skip is built if the check ever
    fails).
  * counts are integers <= 512, exact in fp16; tables are fp16 (the
    ~2^-11 relative table quantization gives ~1e-3 end-to-end error,
    far inside the 2e-2 gate).
  * the output is written as fp16 and upcast on the host, halving the
    dominant HBM write stream.

Sharding: data-parallel over batch. 32 batches / 8 cores = 4 per core.
Per-core device work: PE = (1+2+2+2 group passes/batch * 4 batches) *
4000 cols = 112k fp16 columns ~ 46.7us (the critical engine, ~92% busy);
output DMA = 64 fp16 chunk writes alternated over the SP and Pool DGE
queues (~21us each, ~40us with input tables); PSUM->SBUF fp16 copies
alternate DVE/ACT (~41/40us).  One warm-up matmul on a zeroed tile at
t=0 paces the pipeline while the first tables stream in.  Measured
CoreSim kernel time ~52.5us = ~2us first-table latency + 46.7us gapless
full-clock PE + ~3.8us drain (last copy + DGE latency + DMA sem).
"""

import numpy as np

# Problem constants (hardcoded per harness contract).
B, T = 32, 512
K = 110
C = 20
EK = 100
EI = 200
V = 2 * K + 1          # 221 token vocabulary
F = C * EI             # 4000 flattened (C, EI)
NCORES = 8
BPC = B // NCORES      # batches per core = 4
PB = 128               # timesteps per block (partition dim)
NBLK = T // PB         # blocks per batch = 4
S1 = 128               # group-1 slots (slot 0 = init row)
S2 = 96                # group-2 slots (93 max used + padding)
NQ = 4                 # 1000-col output chunks per block

_CACHE = {}


def _build_program8():
    """fp8e4 DoubleRow program: one structural matmul pass per block.

    DoubleRow packs TWO contraction planes per matmul: lhsT [128, 2, M] and
    rhs [128, 2, N] contract over (partition, plane) = 256 slots in one
    instruction at 0.5 cycles per output column.  Slot (p, i) = i*128 + p:
    slot 0 = the per-batch init row (count pinned to 1), slots 1..221 the
    token vocabulary in plain order, the rest zero padding.  Counts are
    small integers (max ~10 << 16) so they are EXACT in e4m3; the update
    table is stored as an e4m3 hi+lo pair (~2^-8 effective mantissa,
    ~2.6e-3 end-to-end error).  PE cost: 16 blocks x 4000 cols x 2 planes
    x 0.5 = 64k cycles ~ 26.7us.
    """
    import concourse.bass as bass
    import concourse.tile as tile
    from concourse import bacc, mybir

    f8 = mybir.dt.float8e4
    f16 = mybir.dt.float16
    f32 = mybir.dt.float32
    DR = mybir.MatmulPerfMode.DoubleRow

    nc = bacc.Bacc("TRN2")

    d_ct = nc.dram_tensor("ct8", [BPC * PB, 2 * T], f8, kind="ExternalInput")
    d_uthi = nc.dram_tensor("uthi", [PB, 2 * F], f8, kind="ExternalInput")
    d_utlo = nc.dram_tensor("utlo", [PB, 2 * F], f8, kind="ExternalInput")
    d_out = nc.dram_tensor("out", [BPC * T, F], f16, kind="ExternalOutput")

    with tile.TileContext(nc) as tc:
        with (
            tc.tile_pool(name="warm", bufs=1) as warmp,
            tc.tile_pool(name="cts", bufs=2) as ctp,
            tc.tile_pool(name="uts", bufs=2) as utp,
            tc.tile_pool(name="stage", bufs=3) as stagep,
            tc.tile_pool(name="mps", bufs=4, space=bass.MemorySpace.PSUM) as mpsp,
        ):
            # ---- PE pacing warm-up while the first tables stream in -------
            wz = warmp.tile([PB, 512], f16)
            nc.gpsimd.memset(wz[:], 0.0)
            wps = mpsp.tile([PB, 2, 512], f32, tag="pb", name="wps")
            nc.tensor.matmul(wps[:, 0, :], wz[:, 0:PB], wz[:, 0:512], start=True, stop=True)

            # ---- shared fp8 table: loaded once, read by every batch ----
            uthi = utp.tile([PB, 2, F], f8, name="uthi")
            utlo = utp.tile([PB, 2, F], f8, name="utlo")
            # first-chunk-critical pieces (cols 0:2000 of both planes of both
            # tables) spread across three queues; the rest follow
            nc.gpsimd.dma_start(uthi[:, 0, 0:2000], d_uthi[:, 0:2000])
            nc.sync.dma_start(uthi[:, 1, 0:2000], d_uthi[:, F : F + 2000])
            nc.sync.dma_start(utlo[:, 0, 0:2000], d_utlo[:, 0:2000])
            nc.gpsimd.dma_start(utlo[:, 1, 0:2000], d_utlo[:, F : F + 2000])
            nc.sync.dma_start(uthi[:, 0, 2000:4000], d_uthi[:, 2000:4000])
            nc.gpsimd.dma_start(uthi[:, 1, 2000:4000], d_uthi[:, F + 2000 :])
            nc.sync.dma_start(utlo[:, 0, 2000:4000], d_utlo[:, 2000:4000])
            nc.gpsimd.dma_start(utlo[:, 1, 2000:4000], d_utlo[:, F + 2000 :])

            def load_batch(b):
                ct = ctp.tile([PB, 2, T], f8, tag="ct", name=f"ct_{b}")
                r = slice(b * PB, (b + 1) * PB)
                nc.sync.dma_start(ct[:], d_ct[r, :])
                return ct

            ct_next = load_batch(0)
            for b in range(BPC):
                ct = ct_next
                if b + 1 < BPC:
                    ct_next = load_batch(b + 1)
                for k in range(NBLK):
                    j = b * NBLK + k
                    ks = slice(k * PB, (k + 1) * PB)
                    # stage as [p, half, 500]: chunk q = halves 2q, 2q+1
                    stage = stagep.tile([PB, 2 * NQ, 500], f16, tag="stage")
                    for q in range(NQ):
                        c0 = q * 1000
                        # one 2-bank PSUM tile [p, bank, col] per 1000-col
                        # chunk; each bank gets its own hi+lo matmul group
                        pb2 = mpsp.tile([PB, 2, 512], f32, tag="pb", name="pb2")
                        for h in (0, 1):
                            cs = slice(c0 + h * 500, c0 + (h + 1) * 500)
                            nc.tensor.matmul(
                                pb2[:, h, 0:500], ct[:, :, ks],
                                uthi[:, :, cs],
                                start=True, stop=False, perf_mode=DR,
                            )
                        for h in (0, 1):
                            cs = slice(c0 + h * 500, c0 + (h + 1) * 500)
                            nc.tensor.matmul(
                                pb2[:, h, 0:500], ct[:, :, ks],
                                utlo[:, :, cs],
                                start=False, stop=True, perf_mode=DR,
                            )
                        rows = slice(j * PB, (j + 1) * PB)
                        hs = slice(2 * q, 2 * q + 2)
                        if j == BPC * NBLK - 1 and q == NQ - 1:
                            # drain tail: split the final chunk per bank and
                            # per queue so the last copy+DMA are short
                            nc.vector.tensor_copy(
                                stage[:, 2 * q, :], pb2[:, 0, 0:500]
                            )
                            nc.scalar.copy(
                                stage[:, 2 * q + 1, :], pb2[:, 1, 0:500]
                            )
                            nc.sync.dma_start(
                                d_out[rows, c0 : c0 + 500], stage[:, 2 * q, :]
                            )
                            nc.gpsimd.dma_start(
                                d_out[rows, c0 + 500 : c0 + 1000],
                                stage[:, 2 * q + 1, :],
                            )
                        else:
                            # one strided pair-copy [128, 2, 500] per chunk
                            # (skips the 12-col inter-bank gap); alternate
                            # engines per chunk
                            if ((j + q) % 2 == 0) != (j == 0 and q == 0):
                                nc.scalar.copy(
                                    stage[:, hs, :], pb2[:, :, 0:500]
                                )
                            else:
                                nc.vector.tensor_copy(
                                    stage[:, hs, :], pb2[:, :, 0:500]
                                )
                            dst = d_out[rows, c0 : c0 + 1000]
                            if (j + q) % 2 == 0:
                                nc.sync.dma_start(dst, stage[:, hs, :])
                            else:
                                nc.gpsimd.dma_start(dst, stage[:, hs, :])

    nc.compile()
    return nc


def _host_inputs8(x, question_emb, interaction_emb, key_memory, value_memory_init):
    """Host prep for the fp8 DoubleRow path.  Returns (in_maps, ok).

    Slot map (shared table per core): slots 0..220 = token vocabulary,
    slots 221..224 = the core's four per-batch init rows (each batch's
    count matrix pins its own init slot to 1), rest zero padding.
    """
    import ml_dtypes

    f8 = ml_dtypes.float8_e4m3
    x = np.asarray(x).astype(np.int64)
    question_emb = np.asarray(question_emb, dtype=np.float32)
    interaction_emb = np.asarray(interaction_emb, dtype=np.float32)
    key_memory = np.asarray(key_memory, dtype=np.float32)
    value_memory_init = np.asarray(value_memory_init, dtype=np.float32)

    v = np.arange(V, dtype=np.int64)
    qid = (v - 1) % K + 1
    logits = question_emb[qid] @ key_memory.T
    logits -= logits.max(axis=1, keepdims=True)
    w = np.exp(logits)
    w /= w.sum(axis=1, keepdims=True)
    tanhe = np.tanh(interaction_emb)
    utab = (w[:, :, None] * tanhe[:, None, :]).reshape(V, F)   # [221, F]

    ok = True
    in_maps = []
    for core in range(NCORES):
        tbl = np.zeros((2 * PB, F), np.float32)
        tbl[0:V] = utab
        for b in range(BPC):
            tbl[V + b] = value_memory_init[core * BPC + b].reshape(F)
        hi = tbl.astype(f8)
        lo = (tbl - hi.astype(np.float32)).astype(f8)
        # [slot, f] -> [p, i, f] -> [p, 2*F]
        uthi_c = hi.reshape(2, PB, F).transpose(1, 0, 2).reshape(PB, 2 * F)
        utlo_c = lo.reshape(2, PB, F).transpose(1, 0, 2).reshape(PB, 2 * F)
        ct8 = np.zeros((BPC * PB, 2 * T), f8)
        for b in range(BPC):
            xb = x[core * BPC + b]
            cum = np.cumsum(xb[:, None] == v[None, :], axis=0)  # [T, V] ints
            if cum[-1].max() > 16:
                ok = False
            ctsl = np.zeros((2 * PB, T), np.float32)
            ctsl[0:V] = cum.T
            ctsl[V + b] = 1.0
            ct8[b * PB : (b + 1) * PB] = (
                ctsl.reshape(2, PB, T).transpose(1, 0, 2).reshape(PB, 2 * T)
            ).astype(f8)
        in_maps.append({"ct8": ct8, "uthi": uthi_c, "utlo": utlo_c})
    return in_maps, ok


def _build_program(skip0: bool):
    import concourse.bass as bass
    import concourse.tile as tile
    from concourse import bacc, mybir

    f16 = mybir.dt.float16
    f32 = mybir.dt.float32

    nc = bacc.Bacc("TRN2")

    d_ct1 = nc.dram_tensor("ct1", [BPC * S1, T], f16, kind="ExternalInput")
    d_ct2 = nc.dram_tensor("ct2", [BPC * S2, T], f16, kind="ExternalInput")
    d_ut1 = nc.dram_tensor("ut1", [BPC * S1, F], f16, kind="ExternalInput")
    d_ut2 = nc.dram_tensor("ut2", [BPC * S2, F], f16, kind="ExternalInput")
    d_out = nc.dram_tensor("out", [BPC * T, F], f16, kind="ExternalOutput")

    with tile.TileContext(nc) as tc:
        with (
            tc.tile_pool(name="warm", bufs=1) as warmp,
            tc.tile_pool(name="cts", bufs=2) as ctp,
            tc.tile_pool(name="uts", bufs=2) as utp,
            tc.tile_pool(name="stage", bufs=3) as stagep,
            tc.tile_pool(name="wps", bufs=1, space=bass.MemorySpace.PSUM) as wpsp,
            tc.tile_pool(name="mps", bufs=7, space=bass.MemorySpace.PSUM) as mpsp,
        ):
            # ---- PE p-state warm-up while the first tables stream in ------
            wz = warmp.tile([PB, 512], f16)
            nc.gpsimd.memset(wz[:], 0.0)
            wps = wpsp.tile([PB, 512], f32)
            for i in range(1):
                nc.tensor.matmul(
                    wps[:], wz[:, 0:PB], wz[:, 0:512],
                    start=True, stop=True,
                )

            # ---- per-batch input streams (double buffered) ----------------
            def load_batch(b):
                ct1 = ctp.tile([S1, T], f16, tag="ct1", name=f"ct1_{b}")
                ct2 = ctp.tile([S2, T], f16, tag="ct2", name=f"ct2_{b}")
                ut1 = utp.tile([S1, F], f16, tag="ut1", name=f"ut1_{b}")
                ut2 = utp.tile([S2, F], f16, tag="ut2", name=f"ut2_{b}")
                r1 = slice(b * S1, (b + 1) * S1)
                r2 = slice(b * S2, (b + 1) * S2)
                # ct1 + the first ut1 piece land first on separate queues so
                # the batch's first (group-1-only) matmul can start ASAP;
                # group-2 inputs follow behind.
                nc.sync.dma_start(ct1[:], d_ct1[r1, :])
                nc.gpsimd.dma_start(ut1[:, 0:500], d_ut1[r1, 0:500])
                nc.sync.dma_start(ut1[:, 500:1000], d_ut1[r1, 500:1000])
                for q in range(1, NQ):
                    qs = slice(q * 1000, (q + 1) * 1000)
                    if q % 2 == 0:
                        nc.sync.dma_start(ut1[:, qs], d_ut1[r1, qs])
                    else:
                        nc.gpsimd.dma_start(ut1[:, qs], d_ut1[r1, qs])
                nc.gpsimd.dma_start(ct2[:], d_ct2[r2, :])
                for q in range(NQ):
                    qs = slice(q * 1000, (q + 1) * 1000)
                    if q % 2 == 0:
                        nc.gpsimd.dma_start(ut2[:, qs], d_ut2[r2, qs])
                    else:
                        nc.sync.dma_start(ut2[:, qs], d_ut2[r2, qs])
                return ct1, ct2, ut1, ut2

            tiles = load_batch(0)
            for b in range(BPC):
                ct1, ct2, ut1, ut2 = tiles
                if b + 1 < BPC:
                    tiles = load_batch(b + 1)
                for k in range(NBLK):
                    j = b * NBLK + k
                    ks = slice(k * PB, (k + 1) * PB)
                    two_groups = (k > 0) or not skip0
                    stage = stagep.tile([PB, F], f16, tag="stage")
                    for q in range(NQ):
                        c0 = q * 1000
                        pba = mpsp.tile([PB, 512], f32, tag="pb", name="pba")
                        pbb = mpsp.tile([PB, 512], f32, tag="pb", name="pbb")
                        pair = ((pba, c0), (pbb, c0 + 500))
                        for pb_, c in pair:
                            nc.tensor.matmul(
                                pb_[:, 0:500], ct1[:, ks], ut1[:, c : c + 500],
                                start=True, stop=not two_groups,
                            )
                        if two_groups:
                            for pb_, c in pair:
                                nc.tensor.matmul(
                                    pb_[:, 0:500], ct2[:, ks],
                                    ut2[:, c : c + 500],
                                    start=False, stop=True,
                                )
                        nc.vector.tensor_copy(
                            stage[:, c0 : c0 + 500], pba[:, 0:500]
                        )
                        nc.scalar.copy(
                            stage[:, c0 + 500 : c0 + 1000], pbb[:, 0:500]
                        )
                        rows = slice(j * PB, (j + 1) * PB)
                        if j == BPC * NBLK - 1 and q == NQ - 1:
                            # shorten the drain tail: the final chunk leaves
                            # as two parallel 500-col DMAs
                            nc.sync.dma_start(
                                d_out[rows, c0 : c0 + 500],
                                stage[:, c0 : c0 + 500],
                            )
                            nc.gpsimd.dma_start(
                                d_out[rows, c0 + 500 : c0 + 1000],
                                stage[:, c0 + 500 : c0 + 1000],
                            )
                        else:
                            dst = d_out[rows, c0 : c0 + 1000]
                            if (j + q) % 2 == 0:
                                nc.sync.dma_start(dst, stage[:, c0 : c0 + 1000])
                            else:
                                nc.gpsimd.dma_start(dst, stage[:, c0 : c0 + 1000])

    nc.compile()
    return nc


def _host_inputs(x, question_emb, interaction_emb, key_memory, value_memory_init):
    """Host-side precompute: update table, per-batch first-use slot maps,
    cumulative counts, and fp16 shards.  Returns (in_maps, skip0_ok)."""
    x = np.asarray(x).astype(np.int64)
    question_emb = np.asarray(question_emb, dtype=np.float32)
    interaction_emb = np.asarray(interaction_emb, dtype=np.float32)
    key_memory = np.asarray(key_memory, dtype=np.float32)
    value_memory_init = np.asarray(value_memory_init, dtype=np.float32)

    v = np.arange(V, dtype=np.int64)
    qid = (v - 1) % K + 1
    logits = question_emb[qid] @ key_memory.T               # [V, C]
    logits -= logits.max(axis=1, keepdims=True)
    w = np.exp(logits)
    w /= w.sum(axis=1, keepdims=True)                       # [V, C]
    tanhe = np.tanh(interaction_emb)                        # [V, EI]
    utab = (w[:, :, None] * tanhe[:, None, :]).reshape(V, F)
    utab16 = utab.astype(np.float16)                        # [V, F]

    SL = S1 + S2
    skip0_ok = True
    in_maps = []
    for core in range(NCORES):
        ct1 = np.zeros((BPC * S1, T), np.float16)
        ct2 = np.zeros((BPC * S2, T), np.float16)
        ut1 = np.zeros((BPC * S1, F), np.float16)
        ut2 = np.zeros((BPC * S2, F), np.float16)
        for b in range(BPC):
            xb = x[core * BPC + b]                          # [T]
            # order tokens by first use
            _, first_pos = np.unique(xb, return_index=True)
            order = xb[np.sort(first_pos)]                  # [D] token ids
            d = len(order)
            if len(np.unique(xb[:PB])) > S1 - 1:
                skip0_ok = False
            ct = np.zeros((SL, T), np.float16)
            ct[0] = 1.0
            cum = np.cumsum(xb[:, None] == order[None, :], axis=0)  # [T, D]
            ct[1 : 1 + d] = cum.T
            ut = np.zeros((SL, F), np.float16)
            ut[0] = value_memory_init[core * BPC + b].reshape(F)
            ut[1 : 1 + d] = utab16[order]
            ct1[b * S1 : (b + 1) * S1] = ct[0:S1]
            ct2[b * S2 : (b + 1) * S2] = ct[S1:SL]
            ut1[b * S1 : (b + 1) * S1] = ut[0:S1]
            ut2[b * S2 : (b + 1) * S2] = ut[S1:SL]
        in_maps.append({"ct1": ct1, "ct2": ct2, "ut1": ut1, "ut2": ut2})
    return in_maps, skip0_ok


def kernel(
    x,
    next_question,
    question_emb,
    interaction_emb,
    key_memory,
    value_memory_init,
):
    from concourse.bass_utils import run_bass_kernel_spmd

    in_maps, ok8 = _host_inputs8(
        x, question_emb, interaction_emb, key_memory, value_memory_init
    )
    if ok8:
        if "nc8" not in _CACHE:
            _CACHE["nc8"] = _build_program8()
        nc = _CACHE["nc8"]
    else:
        # some token count exceeds the e4m3-exact integer range (16):
        # fall back to the fp16 two-group program
        in_maps, skip0_ok = _host_inputs(
            x, question_emb, interaction_emb, key_memory, value_memory_init
        )
        key = ("nc", skip0_ok)
        if key not in _CACHE:
            _CACHE[key] = _build_program(skip0=skip0_ok)
        nc = _CACHE[key]

    res = run_bass_kernel_spmd(nc, in_maps, list(range(NCORES)))
    out = np.concatenate(
        [
            np.asarray(r["out"]).astype(np.float32).reshape(BPC, T, C, EI)
            for r in res.results
        ],
        axis=0,
    )
    return out


# revision 23
# speedup vs baseline: 1.0158x; 1.0158x over previous
"""Trainium2 Bass kernel for the scatter_memory recurrent MemoryBlock problem.

Reference computation (per batch b):
    qid    = (x - 1) % K + 1
    q      = question_emb[qid]                       # [T, EK]
    inter  = tanh(interaction_emb[x])                # [T, EI]
    w      = softmax(q @ key_memory.T)               # [T, C]
    out[t] = value_memory_init + sum_{s<=t} w[s] (x) inter[s]   # [T, C, EI]

Every per-token quantity depends only on the token id x[t] in [0, 220], so
the rank-1 update for token value v is a fixed table row
    U[v] = softmax(QG[v] @ keyT) (x) tanh(E[v])          # [221, 4000]
and out[t] = init + sum_v Counts[t, v] * U[v] with Counts the cumulative
one-hot count matrix.  Both U (221 x 4020 flops of softmax/tanh/outer) and
Counts (a cumulative histogram of x) are tiny and data-independent of the
heavy math, so they are precomputed on the host; the device kernel is the
actual heavy contraction
    out[t, f] = sum_v CT_b[v, t] * UT_b[v, f]            # per batch
which is 99.8% of the reference FLOPs, plus the 256 MB output stream.

Layout tricks (per batch, host side):
  * vocab slots are ordered by FIRST USE in that batch, slot 0 = the init
    row (count pinned to 1).  Slots split into group 1 (128 rows) and
    group 2 (96 rows, zero-padded).  Because t < 128 can touch at most
    128 distinct tokens, block 0 of each batch provably has all-zero
    group-2 counts and its second matmul group is skipped (checked on the
    host; a fallback program without the skip is built if the check ever
    fails).
  * counts are integers <= 512, exact in fp16; tables are fp16 (the
    ~2^-11 relative table quantization gives ~1e-3 end-to-end error,
    far inside the 2e-2 gate).
  * the output is written as fp16 and upcast on the host, halving the
    dominant HBM write stream.

Sharding: data-parallel over batch. 32 batches / 8 cores = 4 per core.
Per-core device work: PE = (1+2+2+2 group passes/batch * 4 batches) *
4000 cols = 112k fp16 columns ~ 46.7us (the critical engine, ~92% busy);
output DMA = 64 fp16 chunk writes alternated over the SP and Pool DGE
queues (~21us each, ~40us with input tables); PSUM->SBUF fp16 copies
alternate DVE/ACT (~41/40us).  One warm-up matmul on a zeroed tile at
t=0 paces the pipeline while the first tables stream in.  Measured
CoreSim kernel time ~52.5us = ~2us first-table latency + 46.7us gapless
full-clock PE + ~3.8us drain (last copy + DGE latency + DMA sem).
"""

import numpy as np

# Problem constants (hardcoded per harness contract).
B, T = 32, 512
K = 110
C = 20
EK = 100
EI = 200
V = 2 * K + 1          # 221 token vocabulary
F = C * EI             # 4000 flattened (C, EI)
NCORES = 8
BPC = B // NCORES      # batches per core = 4
PB = 128               # timesteps per block (partition dim)
NBLK = T // PB         # blocks per batch = 4
S1 = 128               # group-1 slots (slot 0 = init row)
S2 = 96                # group-2 slots (93 max used + padding)
NQ = 4                 # 1000-col output chunks per block

_CACHE = {}


def _build_program8():
    """fp8e4 DoubleRow program: one structural matmul pass per block.

    DoubleRow packs TWO contraction planes per matmul: lhsT [128, 2, M] and
    rhs [128, 2, N] contract over (partition, plane) = 256 slots in one
    instruction at 0.5 cycles per output column.  Slot (p, i) = i*128 + p:
    slot 0 = the per-batch init row (count pinned to 1), slots 1..221 the
    token vocabulary in plain order, the rest zero padding.  Counts are
    small integers (max ~10 << 16) so they are EXACT in e4m3; the update
    table is stored as an e4m3 hi+lo pair (~2^-8 effective mantissa,
    ~2.6e-3 end-to-end error).  PE cost: 16 blocks x 4000 cols x 2 planes
    x 0.5 = 64k cycles ~ 26.7us.
    """
    import concourse.bass as bass
    import concourse.tile as tile
    from concourse import bacc, mybir

    f8 = mybir.dt.float8e4
    f16 = mybir.dt.float16
    f32 = mybir.dt.float32
    DR = mybir.MatmulPerfMode.DoubleRow

    nc = bacc.Bacc("TRN2")

    d_ct = nc.dram_tensor("ct8", [BPC * PB, 2 * T], f8, kind="ExternalInput")
    d_uthi = nc.dram_tensor("uthi", [PB, 2 * F], f8, kind="ExternalInput")
    d_utlo = nc.dram_tensor("utlo", [PB, 2 * F], f8, kind="ExternalInput")
    d_out = nc.dram_tensor("out", [BPC * T, F], f16, kind="ExternalOutput")

    with tile.TileContext(nc) as tc:
        with (
            tc.tile_pool(name="warm", bufs=1) as warmp,
            tc.tile_pool(name="cts", bufs=2) as ctp,
            tc.tile_pool(name="uts", bufs=2) as utp,
            tc.tile_pool(name="stage", bufs=3) as stagep,
            tc.tile_pool(name="mps", bufs=4, space=bass.MemorySpace.PSUM) as mpsp,
        ):
            # ---- PE pacing warm-up while the first tables stream in -------
            wz = warmp.tile([PB, 512], f16)
            nc.gpsimd.memset(wz[:], 0.0)
            wps = mpsp.tile([PB, 2, 512], f32, tag="pb", name="wps")
            nc.tensor.matmul(wps[:, 0, :], wz[:, 0:PB], wz[:, 0:512], start=True, stop=True)

            # ---- shared fp8 table: loaded once, read by every batch ----
            uthi = utp.tile([PB, 2, F], f8, name="uthi")
            utlo = utp.tile([PB, 2, F], f8, name="utlo")
            # first-chunk-critical pieces (cols 0:2000 of both planes of both
            # tables) spread across three queues; the rest follow
            nc.gpsimd.dma_start(uthi[:, 0, 0:2000], d_uthi[:, 0:2000])
            nc.sync.dma_start(uthi[:, 1, 0:2000], d_uthi[:, F : F + 2000])
            nc.scalar.dma_start(utlo[:, 0, 0:2000], d_utlo[:, 0:2000])
            nc.gpsimd.dma_start(utlo[:, 1, 0:2000], d_utlo[:, F : F + 2000])
            nc.sync.dma_start(uthi[:, 0, 2000:4000], d_uthi[:, 2000:4000])
            nc.gpsimd.dma_start(uthi[:, 1, 2000:4000], d_uthi[:, F + 2000 :])
            nc.sync.dma_start(utlo[:, 0, 2000:4000], d_utlo[:, 2000:4000])
            nc.gpsimd.dma_start(utlo[:, 1, 2000:4000], d_utlo[:, F + 2000 :])

            def load_batch(b):
                ct = ctp.tile([PB, 2, T], f8, tag="ct", name=f"ct_{b}")
                r = slice(b * PB, (b + 1) * PB)
                nc.sync.dma_start(ct[:], d_ct[r, :])
                return ct

            ct_next = load_batch(0)
            for b in range(BPC):
                ct = ct_next
                if b + 1 < BPC:
                    ct_next = load_batch(b + 1)
                for k in range(NBLK):
                    j = b * NBLK + k
                    ks = slice(k * PB, (k + 1) * PB)
                    # stage as [p, half, 500]: chunk q = halves 2q, 2q+1
                    stage = stagep.tile([PB, 2 * NQ, 500], f16, tag="stage")
                    for q in range(NQ):
                        c0 = q * 1000
                        # one 2-bank PSUM tile [p, bank, col] per 1000-col
                        # chunk; each bank gets its own hi+lo matmul group
                        pb2 = mpsp.tile([PB, 2, 512], f32, tag="pb", name="pb2")
                        for h in (0, 1):
                            cs = slice(c0 + h * 500, c0 + (h + 1) * 500)
                            nc.tensor.matmul(
                                pb2[:, h, 0:500], ct[:, :, ks],
                                uthi[:, :, cs],
                                start=True, stop=False, perf_mode=DR,
                            )
                        for h in (0, 1):
                            cs = slice(c0 + h * 500, c0 + (h + 1) * 500)
                            nc.tensor.matmul(
                                pb2[:, h, 0:500], ct[:, :, ks],
                                utlo[:, :, cs],
                                start=False, stop=True, perf_mode=DR,
                            )
                        rows = slice(j * PB, (j + 1) * PB)
                        hs = slice(2 * q, 2 * q + 2)
                        if j == BPC * NBLK - 1 and q == NQ - 1:
                            # drain tail: split the final chunk per bank and
                            # per queue so the last copy+DMA are short
                            nc.vector.tensor_copy(
                                stage[:, 2 * q, :], pb2[:, 0, 0:500]
                            )
                            nc.scalar.copy(
                                stage[:, 2 * q + 1, :], pb2[:, 1, 0:500]
                            )
                            nc.sync.dma_start(
                                d_out[rows, c0 : c0 + 500], stage[:, 2 * q, :]
                            )
                            nc.gpsimd.dma_start(
                                d_out[rows, c0 + 500 : c0 + 1000],
                                stage[:, 2 * q + 1, :],
                            )
                        else:
                            # one strided pair-copy [128, 2, 500] per chunk
                            # (skips the 12-col inter-bank gap); alternate
                            # engines per chunk
                            to_act = (j + q) % 2 == 0 or (q == 3 and j in (1, 5, 9))
                            if to_act and not (j == 0 and q == 0):
                                nc.scalar.copy(
                                    stage[:, hs, :], pb2[:, :, 0:500]
                                )
                            else:
                                nc.vector.tensor_copy(
                                    stage[:, hs, :], pb2[:, :, 0:500]
                                )
                            dst = d_out[rows, c0 : c0 + 1000]
                            if (j + q) % 2 == 0:
                                nc.sync.dma_start(dst, stage[:, hs, :])
                            else:
                                nc.gpsimd.dma_start(dst, stage[:, hs, :])

    nc.compile()
    return nc


def _host_inputs8(x, question_emb, interaction_emb, key_memory, value_memory_init):
    """Host prep for the fp8 DoubleRow path.  Returns (in_maps, ok).

    Slot map (shared table per core): slots 0..220 = token vocabulary,
    slots 221..224 = the core's four per-batch init rows (each batch's
    count matrix pins its own init slot to 1), rest zero padding.
    """
    import ml_dtypes

    f8 = ml_dtypes.float8_e4m3
    x = np.asarray(x).astype(np.int64)
    question_emb = np.asarray(question_emb, dtype=np.float32)
    interaction_emb = np.asarray(interaction_emb, dtype=np.float32)
    key_memory = np.asarray(key_memory, dtype=np.float32)
    value_memory_init = np.asarray(value_memory_init, dtype=np.float32)

    v = np.arange(V, dtype=np.int64)
    qid = (v - 1) % K + 1
    logits = question_emb[qid] @ key_memory.T
    logits -= logits.max(axis=1, keepdims=True)
    w = np.exp(logits)
    w /= w.sum(axis=1, keepdims=True)
    tanhe = np.tanh(interaction_emb)
    utab = (w[:, :, None] * tanhe[:, None, :]).reshape(V, F)   # [221, F]

    ok = True
    in_maps = []
    for core in range(NCORES):
        tbl = np.zeros((2 * PB, F), np.float32)
        tbl[0:V] = utab
        for b in range(BPC):
            tbl[V + b] = value_memory_init[core * BPC + b].reshape(F)
        hi = tbl.astype(f8)
        lo = (tbl - hi.astype(np.float32)).astype(f8)
        # [slot, f] -> [p, i, f] -> [p, 2*F]
        uthi_c = hi.reshape(2, PB, F).transpose(1, 0, 2).reshape(PB, 2 * F)
        utlo_c = lo.reshape(2, PB, F).transpose(1, 0, 2).reshape(PB, 2 * F)
        ct8 = np.zeros((BPC * PB, 2 * T), f8)
        for b in range(BPC):
            xb = x[core * BPC + b]
            cum = np.cumsum(xb[:, None] == v[None, :], axis=0)  # [T, V] ints
            if cum[-1].max() > 16:
                ok = False
            ctsl = np.zeros((2 * PB, T), np.float32)
            ctsl[0:V] = cum.T
            ctsl[V + b] = 1.0
            ct8[b * PB : (b + 1) * PB] = (
                ctsl.reshape(2, PB, T).transpose(1, 0, 2).reshape(PB, 2 * T)
            ).astype(f8)
        in_maps.append({"ct8": ct8, "uthi": uthi_c, "utlo": utlo_c})
    return in_maps, ok


def _build_program(skip0: bool):
    import concourse.bass as bass
    import concourse.tile as tile
    from concourse import bacc, mybir

    f16 = mybir.dt.float16
    f32 = mybir.dt.float32

    nc = bacc.Bacc("TRN2")

    d_ct1 = nc.dram_tensor("ct1", [BPC * S1, T], f16, kind="ExternalInput")
    d_ct2 = nc.dram_tensor("ct2", [BPC * S2, T], f16, kind="ExternalInput")
    d_ut1 = nc.dram_tensor("ut1", [BPC * S1, F], f16, kind="ExternalInput")
    d_ut2 = nc.dram_tensor("ut2", [BPC * S2, F], f16, kind="ExternalInput")
    d_out = nc.dram_tensor("out", [BPC * T, F], f16, kind="ExternalOutput")

    with tile.TileContext(nc) as tc:
        with (
            tc.tile_pool(name="warm", bufs=1) as warmp,
            tc.tile_pool(name="cts", bufs=2) as ctp,
            tc.tile_pool(name="uts", bufs=2) as utp,
            tc.tile_pool(name="stage", bufs=3) as stagep,
            tc.tile_pool(name="wps", bufs=1, space=bass.MemorySpace.PSUM) as wpsp,
            tc.tile_pool(name="mps", bufs=7, space=bass.MemorySpace.PSUM) as mpsp,
        ):
            # ---- PE p-state warm-up while the first tables stream in ------
            wz = warmp.tile([PB, 512], f16)
            nc.gpsimd.memset(wz[:], 0.0)
            wps = wpsp.tile([PB, 512], f32)
            for i in range(1):
                nc.tensor.matmul(
                    wps[:], wz[:, 0:PB], wz[:, 0:512],
                    start=True, stop=True,
                )

            # ---- per-batch input streams (double buffered) ----------------
            def load_batch(b):
                ct1 = ctp.tile([S1, T], f16, tag="ct1", name=f"ct1_{b}")
                ct2 = ctp.tile([S2, T], f16, tag="ct2", name=f"ct2_{b}")
                ut1 = utp.tile([S1, F], f16, tag="ut1", name=f"ut1_{b}")
                ut2 = utp.tile([S2, F], f16, tag="ut2", name=f"ut2_{b}")
                r1 = slice(b * S1, (b + 1) * S1)
                r2 = slice(b * S2, (b + 1) * S2)
                # ct1 + the first ut1 piece land first on separate queues so
                # the batch's first (group-1-only) matmul can start ASAP;
                # group-2 inputs follow behind.
                nc.sync.dma_start(ct1[:], d_ct1[r1, :])
                nc.gpsimd.dma_start(ut1[:, 0:500], d_ut1[r1, 0:500])
                nc.sync.dma_start(ut1[:, 500:1000], d_ut1[r1, 500:1000])
                for q in range(1, NQ):
                    qs = slice(q * 1000, (q + 1) * 1000)
                    if q % 2 == 0:
                        nc.sync.dma_start(ut1[:, qs], d_ut1[r1, qs])
                    else:
                        nc.gpsimd.dma_start(ut1[:, qs], d_ut1[r1, qs])
                nc.gpsimd.dma_start(ct2[:], d_ct2[r2, :])
                for q in range(NQ):
                    qs = slice(q * 1000, (q + 1) * 1000)
                    if q % 2 == 0:
                        nc.gpsimd.dma_start(ut2[:, qs], d_ut2[r2, qs])
                    else:
                        nc.sync.dma_start(ut2[:, qs], d_ut2[r2, qs])
                return ct1, ct2, ut1, ut2

            tiles = load_batch(0)
            for b in range(BPC):
                ct1, ct2, ut1, ut2 = tiles
                if b + 1 < BPC:
                    tiles = load_batch(b + 1)
                for k in range(NBLK):
                    j = b * NBLK + k
                    ks = slice(k * PB, (k + 1) * PB)
                    two_groups = (k > 0) or not skip0
                    stage = stagep.tile([PB, F], f16, tag="stage")
                    for q in range(NQ):
                        c0 = q * 1000
                        pba = mpsp.tile([PB, 512], f32, tag="pb", name="pba")
                        pbb = mpsp.tile([PB, 512], f32, tag="pb", name="pbb")
                        pair = ((pba, c0), (pbb, c0 + 500))
                        for pb_, c in pair:
                            nc.tensor.matmul(
                                pb_[:, 0:500], ct1[:, ks], ut1[:, c : c + 500],
                                start=True, stop=not two_groups,
                            )
                        if two_groups:
                            for pb_, c in pair:
                                nc.tensor.matmul(
                                    pb_[:, 0:500], ct2[:, ks],
                                    ut2[:, c : c + 500],
                                    start=False, stop=True,
                                )
                        nc.vector.tensor_copy(
                            stage[:, c0 : c0 + 500], pba[:, 0:500]
                        )
                        nc.scalar.copy(
                            stage[:, c0 + 500 : c0 + 1000], pbb[:, 0:500]
                        )
                        rows = slice(j * PB, (j + 1) * PB)
                        if j == BPC * NBLK - 1 and q == NQ - 1:
                            # shorten the drain tail: the final chunk leaves
                            # as two parallel 500-col DMAs
                            nc.sync.dma_start(
                                d_out[rows, c0 : c0 + 500],
                                stage[:, c0 : c0 + 500],
                            )
                            nc.gpsimd.dma_start(
                                d_out[rows, c0 + 500 : c0 + 1000],
                                stage[:, c0 + 500 : c0 + 1000],
                            )
                        else:
                            dst = d_out[rows, c0 : c0 + 1000]
                            if (j + q) % 2 == 0:
                                nc.sync.dma_start(dst, stage[:, c0 : c0 + 1000])
                            else:
                                nc.gpsimd.dma_start(dst, stage[:, c0 : c0 + 1000])

    nc.compile()
    return nc


def _host_inputs(x, question_emb, interaction_emb, key_memory, value_memory_init):
    """Host-side precompute: update table, per-batch first-use slot maps,
    cumulative counts, and fp16 shards.  Returns (in_maps, skip0_ok)."""
    x = np.asarray(x).astype(np.int64)
    question_emb = np.asarray(question_emb, dtype=np.float32)
    interaction_emb = np.asarray(interaction_emb, dtype=np.float32)
    key_memory = np.asarray(key_memory, dtype=np.float32)
    value_memory_init = np.asarray(value_memory_init, dtype=np.float32)

    v = np.arange(V, dtype=np.int64)
    qid = (v - 1) % K + 1
    logits = question_emb[qid] @ key_memory.T               # [V, C]
    logits -= logits.max(axis=1, keepdims=True)
    w = np.exp(logits)
    w /= w.sum(axis=1, keepdims=True)                       # [V, C]
    tanhe = np.tanh(interaction_emb)                        # [V, EI]
    utab = (w[:, :, None] * tanhe[:, None, :]).reshape(V, F)
    utab16 = utab.astype(np.float16)                        # [V, F]

    SL = S1 + S2
    skip0_ok = True
    in_maps = []
    for core in range(NCORES):
        ct1 = np.zeros((BPC * S1, T), np.float16)
        ct2 = np.zeros((BPC * S2, T), np.float16)
        ut1 = np.zeros((BPC * S1, F), np.float16)
        ut2 = np.zeros((BPC * S2, F), np.float16)
        for b in range(BPC):
            xb = x[core * BPC + b]                          # [T]
            # order tokens by first use
            _, first_pos = np.unique(xb, return_index=True)
            order = xb[np.sort(first_pos)]                  # [D] token ids
            d = len(order)
            if len(np.unique(xb[:PB])) > S1 - 1:
                skip0_ok = False
            ct = np.zeros((SL, T), np.float16)
            ct[0] = 1.0
            cum = np.cumsum(xb[:, None] == order[None, :], axis=0)  # [T, D]
            ct[1 : 1 + d] = cum.T
            ut = np.zeros((SL, F), np.float16)
            ut[0] = value_memory_init[core * BPC + b].reshape(F)
            ut[1 : 1 + d] = utab16[order]
            ct1[b * S1 : (b + 1) * S1] = ct[0:S1]
            ct2[b * S2 : (b + 1) * S2] = ct[S1:SL]
            ut1[b * S1 : (b + 1) * S1] = ut[0:S1]
            ut2[b * S2 : (b + 1) * S2] = ut[S1:SL]
        in_maps.append({"ct1": ct1, "ct2": ct2, "ut1": ut1, "ut2": ut2})
    return in_maps, skip0_ok


def kernel(
    x,
    next_question,
    question_emb,
    interaction_emb,
    key_memory,
    value_memory_init,
):
    from concourse.bass_utils import run_bass_kernel_spmd

    in_maps, ok8 = _host_inputs8(
        x, question_emb, interaction_emb, key_memory, value_memory_init
    )
    if ok8:
        if "nc8" not in _CACHE:
            _CACHE["nc8"] = _build_program8()
        nc = _CACHE["nc8"]
    else:
        # some token count exceeds the e4m3-exact integer range (16):
        # fall back to the fp16 two-group program
        in_maps, skip0_ok = _host_inputs(
            x, question_emb, interaction_emb, key_memory, value_memory_init
        )
        key = ("nc", skip0_ok)
        if key not in _CACHE:
            _CACHE[key] = _build_program(skip0=skip0_ok)
        nc = _CACHE[key]

    res = run_bass_kernel_spmd(nc, in_maps, list(range(NCORES)))
    out = np.concatenate(
        [
            np.asarray(r["out"]).astype(np.float32).reshape(BPC, T, C, EI)
            for r in res.results
        ],
        axis=0,
    )
    return out


# revision 24
# speedup vs baseline: 1.0258x; 1.0099x over previous
"""Trainium2 Bass kernel for the scatter_memory recurrent MemoryBlock problem.

Reference computation (per batch b):
    qid    = (x - 1) % K + 1
    q      = question_emb[qid]                       # [T, EK]
    inter  = tanh(interaction_emb[x])                # [T, EI]
    w      = softmax(q @ key_memory.T)               # [T, C]
    out[t] = value_memory_init + sum_{s<=t} w[s] (x) inter[s]   # [T, C, EI]

Every per-token quantity depends only on the token id x[t] in [0, 220], so
the rank-1 update for token value v is a fixed table row
    U[v] = softmax(QG[v] @ keyT) (x) tanh(E[v])          # [221, 4000]
and out[t] = init + sum_v Counts[t, v] * U[v] with Counts the cumulative
one-hot count matrix.  Both U (221 x 4020 flops of softmax/tanh/outer) and
Counts (a cumulative histogram of x) are tiny and data-independent of the
heavy math, so they are precomputed on the host; the device kernel is the
actual heavy contraction
    out[t, f] = sum_v CT_b[v, t] * UT_b[v, f]            # per batch
which is 99.8% of the reference FLOPs, plus the 256 MB output stream.

Layout tricks (per batch, host side):
  * vocab slots are ordered by FIRST USE in that batch, slot 0 = the init
    row (count pinned to 1).  Slots split into group 1 (128 rows) and
    group 2 (96 rows, zero-padded).  Because t < 128 can touch at most
    128 distinct tokens, block 0 of each batch provably has all-zero
    group-2 counts and its second matmul group is skipped (checked on the
    host; a fallback program without the skip is built if the check ever
    fails).
  * counts are integers <= 512, exact in fp16; tables are fp16 (the
    ~2^-11 relative table quantization gives ~1e-3 end-to-end error,
    far inside the 2e-2 gate).
  * the output is written as fp16 and upcast on the host, halving the
    dominant HBM write stream.

Sharding: data-parallel over batch. 32 batches / 8 cores = 4 per core.
Per-core device work: PE = (1+2+2+2 group passes/batch * 4 batches) *
4000 cols = 112k fp16 columns ~ 46.7us (the critical engine, ~92% busy);
output DMA = 64 fp16 chunk writes alternated over the SP and Pool DGE
queues (~21us each, ~40us with input tables); PSUM->SBUF fp16 copies
alternate DVE/ACT (~41/40us).  One warm-up matmul on a zeroed tile at
t=0 paces the pipeline while the first tables stream in.  Measured
CoreSim kernel time ~52.5us = ~2us first-table latency + 46.7us gapless
full-clock PE + ~3.8us drain (last copy + DGE latency + DMA sem).
"""

import numpy as np

# Problem constants (hardcoded per harness contract).
B, T = 32, 512
K = 110
C = 20
EK = 100
EI = 200
V = 2 * K + 1          # 221 token vocabulary
F = C * EI             # 4000 flattened (C, EI)
NCORES = 8
BPC = B // NCORES      # batches per core = 4
PB = 128               # timesteps per block (partition dim)
NBLK = T // PB         # blocks per batch = 4
S1 = 128               # group-1 slots (slot 0 = init row)
S2 = 96                # group-2 slots (93 max used + padding)
NQ = 4                 # 1000-col output chunks per block

_CACHE = {}


def _build_program8():
    """fp8e4 DoubleRow program: one structural matmul pass per block.

    DoubleRow packs TWO contraction planes per matmul: lhsT [128, 2, M] and
    rhs [128, 2, N] contract over (partition, plane) = 256 slots in one
    instruction at 0.5 cycles per output column.  Slot (p, i) = i*128 + p:
    slot 0 = the per-batch init row (count pinned to 1), slots 1..221 the
    token vocabulary in plain order, the rest zero padding.  Counts are
    small integers (max ~10 << 16) so they are EXACT in e4m3; the update
    table is stored as an e4m3 hi+lo pair (~2^-8 effective mantissa,
    ~2.6e-3 end-to-end error).  PE cost: 16 blocks x 4000 cols x 2 planes
    x 0.5 = 64k cycles ~ 26.7us.
    """
    import concourse.bass as bass
    import concourse.tile as tile
    from concourse import bacc, mybir

    f8 = mybir.dt.float8e4
    f16 = mybir.dt.float16
    f32 = mybir.dt.float32
    DR = mybir.MatmulPerfMode.DoubleRow

    nc = bacc.Bacc("TRN2")

    d_ct = nc.dram_tensor("ct8", [BPC * PB, 2 * T], f8, kind="ExternalInput")
    d_uthi = nc.dram_tensor("uthi", [PB, 2 * F], f8, kind="ExternalInput")
    d_utlo = nc.dram_tensor("utlo", [PB, 2 * F], f8, kind="ExternalInput")
    d_out = nc.dram_tensor("out", [BPC * T, F], f16, kind="ExternalOutput")

    with tile.TileContext(nc) as tc:
        with (
            tc.tile_pool(name="warm", bufs=1) as warmp,
            tc.tile_pool(name="cts", bufs=2) as ctp,
            tc.tile_pool(name="uts", bufs=2) as utp,
            tc.tile_pool(name="stage", bufs=3) as stagep,
            tc.tile_pool(name="mps", bufs=4, space=bass.MemorySpace.PSUM) as mpsp,
        ):
            # ---- PE pacing warm-up while the first tables stream in -------
            wz = warmp.tile([PB, 512], f16)
            nc.gpsimd.memset(wz[:], 0.0)
            wps = mpsp.tile([PB, 2, 512], f32, tag="pb", name="wps")
            nc.tensor.matmul(wps[:, 0, :], wz[:, 0:PB], wz[:, 0:512], start=True, stop=True)

            # ---- shared fp8 table: loaded once, read by every batch ----
            uthi = utp.tile([PB, 2, F], f8, name="uthi")
            utlo = utp.tile([PB, 2, F], f8, name="utlo")
            # first-chunk-critical pieces (cols 0:2000 of both planes of both
            # tables) spread across three queues; the rest follow
            nc.gpsimd.dma_start(uthi[:, 0, 0:2000], d_uthi[:, 0:2000])
            nc.sync.dma_start(uthi[:, 1, 0:2000], d_uthi[:, F : F + 2000])
            nc.scalar.dma_start(utlo[:, 0, 0:2000], d_utlo[:, 0:2000])
            nc.gpsimd.dma_start(utlo[:, 1, 0:2000], d_utlo[:, F : F + 2000])
            nc.sync.dma_start(uthi[:, 0, 2000:4000], d_uthi[:, 2000:4000])
            nc.gpsimd.dma_start(uthi[:, 1, 2000:4000], d_uthi[:, F + 2000 :])
            nc.sync.dma_start(utlo[:, 0, 2000:4000], d_utlo[:, 2000:4000])
            nc.gpsimd.dma_start(utlo[:, 1, 2000:4000], d_utlo[:, F + 2000 :])

            def load_batch(b):
                ct = ctp.tile([PB, 2, T], f8, tag="ct", name=f"ct_{b}")
                r = slice(b * PB, (b + 1) * PB)
                nc.sync.dma_start(ct[:], d_ct[r, :])
                return ct

            ct_next = load_batch(0)
            for b in range(BPC):
                ct = ct_next
                if b + 1 < BPC:
                    ct_next = load_batch(b + 1)
                for k in range(NBLK):
                    j = b * NBLK + k
                    ks = slice(k * PB, (k + 1) * PB)
                    # stage as [p, half, 500]: chunk q = halves 2q, 2q+1
                    stage = stagep.tile([PB, 2 * NQ, 500], f16, tag="stage")
                    for q in range(NQ):
                        c0 = q * 1000
                        # one 2-bank PSUM tile [p, bank, col] per 1000-col
                        # chunk; each bank gets its own hi+lo matmul group
                        pb2 = mpsp.tile([PB, 2, 512], f32, tag="pb", name="pb2")
                        for h in (0, 1):
                            cs = slice(c0 + h * 500, c0 + (h + 1) * 500)
                            nc.tensor.matmul(
                                pb2[:, h, 0:500], ct[:, :, ks],
                                uthi[:, :, cs],
                                start=True, stop=False, perf_mode=DR,
                            )
                        for h in (0, 1):
                            cs = slice(c0 + h * 500, c0 + (h + 1) * 500)
                            nc.tensor.matmul(
                                pb2[:, h, 0:500], ct[:, :, ks],
                                utlo[:, :, cs],
                                start=False, stop=True, perf_mode=DR,
                            )
                        rows = slice(j * PB, (j + 1) * PB)
                        hs = slice(2 * q, 2 * q + 2)
                        if j == BPC * NBLK - 1 and q == NQ - 1:
                            # drain tail: split the final chunk per bank and
                            # per queue so the last copy+DMA are short
                            nc.vector.tensor_copy(
                                stage[:, 2 * q, :], pb2[:, 0, 0:500]
                            )
                            nc.scalar.copy(
                                stage[:, 2 * q + 1, :], pb2[:, 1, 0:500]
                            )
                            nc.sync.dma_start(
                                d_out[rows, c0 : c0 + 500], stage[:, 2 * q, :]
                            )
                            nc.gpsimd.dma_start(
                                d_out[rows, c0 + 500 : c0 + 1000],
                                stage[:, 2 * q + 1, :],
                            )
                        else:
                            # one strided pair-copy [128, 2, 500] per chunk
                            # (skips the 12-col inter-bank gap); alternate
                            # engines per chunk
                            to_act = (j + q) % 2 == 0 or (q == 3 and j in (2, 6, 10))
                            if to_act and not (j == 0 and q == 0):
                                nc.scalar.copy(
                                    stage[:, hs, :], pb2[:, :, 0:500]
                                )
                            else:
                                nc.vector.tensor_copy(
                                    stage[:, hs, :], pb2[:, :, 0:500]
                                )
                            dst = d_out[rows, c0 : c0 + 1000]
                            if (j + q) % 2 == 0:
                                nc.sync.dma_start(dst, stage[:, hs, :])
                            else:
                                nc.gpsimd.dma_start(dst, stage[:, hs, :])

    nc.compile()
    return nc


def _host_inputs8(x, question_emb, interaction_emb, key_memory, value_memory_init):
    """Host prep for the fp8 DoubleRow path.  Returns (in_maps, ok).

    Slot map (shared table per core): slots 0..220 = token vocabulary,
    slots 221..224 = the core's four per-batch init rows (each batch's
    count matrix pins its own init slot to 1), rest zero padding.
    """
    import ml_dtypes

    f8 = ml_dtypes.float8_e4m3
    x = np.asarray(x).astype(np.int64)
    question_emb = np.asarray(question_emb, dtype=np.float32)
    interaction_emb = np.asarray(interaction_emb, dtype=np.float32)
    key_memory = np.asarray(key_memory, dtype=np.float32)
    value_memory_init = np.asarray(value_memory_init, dtype=np.float32)

    v = np.arange(V, dtype=np.int64)
    qid = (v - 1) % K + 1
    logits = question_emb[qid] @ key_memory.T
    logits -= logits.max(axis=1, keepdims=True)
    w = np.exp(logits)
    w /= w.sum(axis=1, keepdims=True)
    tanhe = np.tanh(interaction_emb)
    utab = (w[:, :, None] * tanhe[:, None, :]).reshape(V, F)   # [221, F]

    ok = True
    in_maps = []
    for core in range(NCORES):
        tbl = np.zeros((2 * PB, F), np.float32)
        tbl[0:V] = utab
        for b in range(BPC):
            tbl[V + b] = value_memory_init[core * BPC + b].reshape(F)
        hi = tbl.astype(f8)
        lo = (tbl - hi.astype(np.float32)).astype(f8)
        # [slot, f] -> [p, i, f] -> [p, 2*F]
        uthi_c = hi.reshape(2, PB, F).transpose(1, 0, 2).reshape(PB, 2 * F)
        utlo_c = lo.reshape(2, PB, F).transpose(1, 0, 2).reshape(PB, 2 * F)
        ct8 = np.zeros((BPC * PB, 2 * T), f8)
        for b in range(BPC):
            xb = x[core * BPC + b]
            cum = np.cumsum(xb[:, None] == v[None, :], axis=0)  # [T, V] ints
            if cum[-1].max() > 16:
                ok = False
            ctsl = np.zeros((2 * PB, T), np.float32)
            ctsl[0:V] = cum.T
            ctsl[V + b] = 1.0
            ct8[b * PB : (b + 1) * PB] = (
                ctsl.reshape(2, PB, T).transpose(1, 0, 2).reshape(PB, 2 * T)
            ).astype(f8)
        in_maps.append({"ct8": ct8, "uthi": uthi_c, "utlo": utlo_c})
    return in_maps, ok


def _build_program(skip0: bool):
    import concourse.bass as bass
    import concourse.tile as tile
    from concourse import bacc, mybir

    f16 = mybir.dt.float16
    f32 = mybir.dt.float32

    nc = bacc.Bacc("TRN2")

    d_ct1 = nc.dram_tensor("ct1", [BPC * S1, T], f16, kind="ExternalInput")
    d_ct2 = nc.dram_tensor("ct2", [BPC * S2, T], f16, kind="ExternalInput")
    d_ut1 = nc.dram_tensor("ut1", [BPC * S1, F], f16, kind="ExternalInput")
    d_ut2 = nc.dram_tensor("ut2", [BPC * S2, F], f16, kind="ExternalInput")
    d_out = nc.dram_tensor("out", [BPC * T, F], f16, kind="ExternalOutput")

    with tile.TileContext(nc) as tc:
        with (
            tc.tile_pool(name="warm", bufs=1) as warmp,
            tc.tile_pool(name="cts", bufs=2) as ctp,
            tc.tile_pool(name="uts", bufs=2) as utp,
            tc.tile_pool(name="stage", bufs=3) as stagep,
            tc.tile_pool(name="wps", bufs=1, space=bass.MemorySpace.PSUM) as wpsp,
            tc.tile_pool(name="mps", bufs=7, space=bass.MemorySpace.PSUM) as mpsp,
        ):
            # ---- PE p-state warm-up while the first tables stream in ------
            wz = warmp.tile([PB, 512], f16)
            nc.gpsimd.memset(wz[:], 0.0)
            wps = wpsp.tile([PB, 512], f32)
            for i in range(1):
                nc.tensor.matmul(
                    wps[:], wz[:, 0:PB], wz[:, 0:512],
                    start=True, stop=True,
                )

            # ---- per-batch input streams (double buffered) ----------------
            def load_batch(b):
                ct1 = ctp.tile([S1, T], f16, tag="ct1", name=f"ct1_{b}")
                ct2 = ctp.tile([S2, T], f16, tag="ct2", name=f"ct2_{b}")
                ut1 = utp.tile([S1, F], f16, tag="ut1", name=f"ut1_{b}")
                ut2 = utp.tile([S2, F], f16, tag="ut2", name=f"ut2_{b}")
                r1 = slice(b * S1, (b + 1) * S1)
                r2 = slice(b * S2, (b + 1) * S2)
                # ct1 + the first ut1 piece land first on separate queues so
                # the batch's first (group-1-only) matmul can start ASAP;
                # group-2 inputs follow behind.
                nc.sync.dma_start(ct1[:], d_ct1[r1, :])
                nc.gpsimd.dma_start(ut1[:, 0:500], d_ut1[r1, 0:500])
                nc.sync.dma_start(ut1[:, 500:1000], d_ut1[r1, 500:1000])
                for q in range(1, NQ):
                    qs = slice(q * 1000, (q + 1) * 1000)
                    if q % 2 == 0:
                        nc.sync.dma_start(ut1[:, qs], d_ut1[r1, qs])
                    else:
                        nc.gpsimd.dma_start(ut1[:, qs], d_ut1[r1, qs])
                nc.gpsimd.dma_start(ct2[:], d_ct2[r2, :])
                for q in range(NQ):
                    qs = slice(q * 1000, (q + 1) * 1000)
                    if q % 2 == 0:
                        nc.gpsimd.dma_start(ut2[:, qs], d_ut2[r2, qs])
                    else:
                        nc.sync.dma_start(ut2[:, qs], d_ut2[r2, qs])
                return ct1, ct2, ut1, ut2

            tiles = load_batch(0)
            for b in range(BPC):
                ct1, ct2, ut1, ut2 = tiles
                if b + 1 < BPC:
                    tiles = load_batch(b + 1)
                for k in range(NBLK):
                    j = b * NBLK + k
                    ks = slice(k * PB, (k + 1) * PB)
                    two_groups = (k > 0) or not skip0
                    stage = stagep.tile([PB, F], f16, tag="stage")
                    for q in range(NQ):
                        c0 = q * 1000
                        pba = mpsp.tile([PB, 512], f32, tag="pb", name="pba")
                        pbb = mpsp.tile([PB, 512], f32, tag="pb", name="pbb")
                        pair = ((pba, c0), (pbb, c0 + 500))
                        for pb_, c in pair:
                            nc.tensor.matmul(
                                pb_[:, 0:500], ct1[:, ks], ut1[:, c : c + 500],
                                start=True, stop=not two_groups,
                            )
                        if two_groups:
                            for pb_, c in pair:
                                nc.tensor.matmul(
                                    pb_[:, 0:500], ct2[:, ks],
                                    ut2[:, c : c + 500],
                                    start=False, stop=True,
                                )
                        nc.vector.tensor_copy(
                            stage[:, c0 : c0 + 500], pba[:, 0:500]
                        )
                        nc.scalar.copy(
                            stage[:, c0 + 500 : c0 + 1000], pbb[:, 0:500]
                        )
                        rows = slice(j * PB, (j + 1) * PB)
                        if j == BPC * NBLK - 1 and q == NQ - 1:
                            # shorten the drain tail: the final chunk leaves
                            # as two parallel 500-col DMAs
                            nc.sync.dma_start(
                                d_out[rows, c0 : c0 + 500],
                                stage[:, c0 : c0 + 500],
                            )
                            nc.gpsimd.dma_start(
                                d_out[rows, c0 + 500 : c0 + 1000],
                                stage[:, c0 + 500 : c0 + 1000],
                            )
                        else:
                            dst = d_out[rows, c0 : c0 + 1000]
                            if (j + q) % 2 == 0:
                                nc.sync.dma_start(dst, stage[:, c0 : c0 + 1000])
                            else:
                                nc.gpsimd.dma_start(dst, stage[:, c0 : c0 + 1000])

    nc.compile()
    return nc


def _host_inputs(x, question_emb, interaction_emb, key_memory, value_memory_init):
    """Host-side precompute: update table, per-batch first-use slot maps,
    cumulative counts, and fp16 shards.  Returns (in_maps, skip0_ok)."""
    x = np.asarray(x).astype(np.int64)
    question_emb = np.asarray(question_emb, dtype=np.float32)
    interaction_emb = np.asarray(interaction_emb, dtype=np.float32)
    key_memory = np.asarray(key_memory, dtype=np.float32)
    value_memory_init = np.asarray(value_memory_init, dtype=np.float32)

    v = np.arange(V, dtype=np.int64)
    qid = (v - 1) % K + 1
    logits = question_emb[qid] @ key_memory.T               # [V, C]
    logits -= logits.max(axis=1, keepdims=True)
    w = np.exp(logits)
    w /= w.sum(axis=1, keepdims=True)                       # [V, C]
    tanhe = np.tanh(interaction_emb)                        # [V, EI]
    utab = (w[:, :, None] * tanhe[:, None, :]).reshape(V, F)
    utab16 = utab.astype(np.float16)                        # [V, F]

    SL = S1 + S2
    skip0_ok = True
    in_maps = []
    for core in range(NCORES):
        ct1 = np.zeros((BPC * S1, T), np.float16)
        ct2 = np.zeros((BPC * S2, T), np.float16)
        ut1 = np.zeros((BPC * S1, F), np.float16)
        ut2 = np.zeros((BPC * S2, F), np.float16)
        for b in range(BPC):
            xb = x[core * BPC + b]                          # [T]
            # order tokens by first use
            _, first_pos = np.unique(xb, return_index=True)
            order = xb[np.sort(first_pos)]                  # [D] token ids
            d = len(order)
            if len(np.unique(xb[:PB])) > S1 - 1:
                skip0_ok = False
            ct = np.zeros((SL, T), np.float16)
            ct[0] = 1.0
            cum = np.cumsum(xb[:, None] == order[None, :], axis=0)  # [T, D]
            ct[1 : 1 + d] = cum.T
            ut = np.zeros((SL, F), np.float16)
            ut[0] = value_memory_init[core * BPC + b].reshape(F)
            ut[1 : 1 + d] = utab16[order]
            ct1[b * S1 : (b + 1) * S1] = ct[0:S1]
            ct2[b * S2 : (b + 1) * S2] = ct[S1:SL]
            ut1[b * S1 : (b + 1) * S1] = ut[0:S1]
            ut2[b * S2 : (b + 1) * S2] = ut[S1:SL]
        in_maps.append({"ct1": ct1, "ct2": ct2, "ut1": ut1, "ut2": ut2})
    return in_maps, skip0_ok


def kernel(
    x,
    next_question,
    question_emb,
    interaction_emb,
    key_memory,
    value_memory_init,
):
    from concourse.bass_utils import run_bass_kernel_spmd

    in_maps, ok8 = _host_inputs8(
        x, question_emb, interaction_emb, key_memory, value_memory_init
    )
    if ok8:
        if "nc8" not in _CACHE:
            _CACHE["nc8"] = _build_program8()
        nc = _CACHE["nc8"]
    else:
        # some token count exceeds the e4m3-exact integer range (16):
        # fall back to the fp16 two-group program
        in_maps, skip0_ok = _host_inputs(
            x, question_emb, interaction_emb, key_memory, value_memory_init
        )
        key = ("nc", skip0_ok)
        if key not in _CACHE:
            _CACHE[key] = _build_program(skip0=skip0_ok)
        nc = _CACHE[key]

    res = run_bass_kernel_spmd(nc, in_maps, list(range(NCORES)))
    out = np.concatenate(
        [
            np.asarray(r["out"]).astype(np.float32).reshape(BPC, T, C, EI)
            for r in res.results
        ],
        axis=0,
    )
    return out
